# revision 3
# baseline (speedup 1.0000x reference)
"""Autoformer forward on 8 trn2 NeuronCores, pure data parallel over batch.

kernel(**inputs) takes FULL unsharded inputs (x_enc [32,3072,64],
x_mark_enc [32,3072,4], x_dec [32,1024,64], x_mark_dec [32,1024,4],
params pytree) and returns the FULL output [32,1024,1].

Sharding: batch 32 -> 8 shards of 4, one shard per NeuronCore; params
replicated; per-core outputs gathered on host.

The NeuronCore compiler does not support the HLO `fft` op, so the
FFT-based autocorrelation is computed with an exactly-equivalent
two-stage Cooley-Tukey DFT expressed as matmuls (einsum) + twiddle
multiplies, which lower to TensorE matmuls. top_k is replaced by an
iterative max/where argmax (identical result for distinct values), and
the per-sample circular time-delay gather by dynamic_slice on a doubled
buffer (value-doubling == modular roll, as in the reference).
"""

import math

import jax
import jax.numpy as jnp
import numpy as np

B, SEQ_LEN, LABEL_LEN, PRED_LEN = 32, 3072, 512, 1024
ENC_IN, MARK_DIM, D_MODEL, N_HEADS, D_FF = 64, 4, 256, 8, 1024
E_LAYERS, D_LAYERS, MOVING_AVG, C_OUT, FACTOR = 2, 1, 25, 64, 1
N_CORES = 8

_FACTORS = {3072: (64, 48), 1536: (48, 32), 1024: (32, 32)}


def _tabs(L):
    """Constant DFT/twiddle tables for length L = L1*L2 (float32)."""
    L1, L2 = _FACTORS[L]
    t1 = np.arange(L1)
    C1 = np.cos(2 * np.pi * np.outer(t1, t1) / L1).astype(np.float32)
    S1 = np.sin(2 * np.pi * np.outer(t1, t1) / L1).astype(np.float32)
    t2 = np.arange(L2)
    C2 = np.cos(2 * np.pi * np.outer(t2, t2) / L2).astype(np.float32)
    S2 = np.sin(2 * np.pi * np.outer(t2, t2) / L2).astype(np.float32)
    k1 = np.arange(L1)
    Twc = np.cos(2 * np.pi * np.outer(k1, t2) / L).astype(np.float32)
    Tws = np.sin(2 * np.pi * np.outer(k1, t2) / L).astype(np.float32)
    return L1, L2, C1, S1, C2, S2, Twc, Tws


def _fwd_dft(x, L):
    """DFT of real x[..., L] -> (Xr, Xi) indexed [..., k1, k2], k = k1 + L1*k2."""
    L1, L2, C1, S1, C2, S2, Twc, Tws = _tabs(L)
    x2 = x.reshape(x.shape[:-1] + (L1, L2))  # [..., t1, t2]
    # A[k1, t2] = sum_t1 x[t1, t2] * e^{-2pi i t1 k1 / L1}
    Ar = jnp.einsum("...ts,tk->...ks", x2, C1)
    Ai = -jnp.einsum("...ts,tk->...ks", x2, S1)
    # twiddle by e^{-2pi i k1 t2 / L}
    Br = Ar * Twc + Ai * Tws
    Bi = Ai * Twc - Ar * Tws
    # X[k1, k2] = sum_t2 B[k1, t2] * e^{-2pi i t2 k2 / L2}
    Xr = jnp.einsum("...ks,sj->...kj", Br, C2) + jnp.einsum("...ks,sj->...kj", Bi, S2)
    Xi = jnp.einsum("...ks,sj->...kj", Bi, C2) - jnp.einsum("...ks,sj->...kj", Br, S2)
    return Xr, Xi


def _inv_dft_real(Pr, Pi, L):
    """Real part of (1/L) * sum_k P[k] e^{+2pi i k d / L}; P indexed [..., k1, k2].

    Returns m[..., L] with d = d1*L2 + d2.
    """
    L1, L2, C1, S1, C2, S2, Twc, Tws = _tabs(L)
    # G[k1, d2] = sum_k2 P[k1, k2] e^{+2pi i k2 d2 / L2}
    Gr = jnp.einsum("...kj,jd->...kd", Pr, C2) - jnp.einsum("...kj,jd->...kd", Pi, S2)
    Gi = jnp.einsum("...kj,jd->...kd", Pi, C2) + jnp.einsum("...kj,jd->...kd", Pr, S2)
    # twiddle e^{+2pi i k1 d2 / L}
    Hr = Gr * Twc - Gi * Tws
    Hi = Gi * Twc + Gr * Tws
    # m[d1, d2] = Re sum_k1 H[k1, d2] e^{+2pi i k1 d1 / L1}
    Mr = jnp.einsum("...kd,ke->...ed", Hr, C1) - jnp.einsum("...kd,ke->...ed", Hi, S1)
    out = Mr.reshape(Mr.shape[:-2] + (L,)) / L
    return out


def moving_avg(x, k):
    p = (k - 1) // 2
    xp = jnp.concatenate(
        [jnp.repeat(x[:, :1], p, axis=1), x, jnp.repeat(x[:, -1:], p, axis=1)], axis=1
    )
    c = jnp.cumsum(xp, axis=1)
    c = jnp.concatenate([jnp.zeros_like(c[:, :1]), c], axis=1)
    return (c[:, k:] - c[:, :-k]) / k


def series_decomp(x, k):
    m = moving_avg(x, k)
    return x - m, m


def conv1d(x, w, b=None, pad=0):
    if pad:
        x = jnp.concatenate([x[:, -pad:], x, x[:, :pad]], axis=1)
    y = jax.lax.conv_general_dilated(
        x, w, (1,), "VALID", dimension_numbers=("NWC", "WIO", "NWC")
    )
    return y + b if b is not None else y


def autocorrelation(q, k, v, factor):
    Bq, L, H, E = q.shape
    S = k.shape[1]
    if L > S:
        pad = ((0, 0), (0, L - S), (0, 0), (0, 0))
        k = jnp.pad(k, pad)
        v = jnp.pad(v, pad)
    else:
        k = k[:, :L]
        v = v[:, :L]
    C = H * E
    qt = jnp.transpose(q, (0, 2, 3, 1)).reshape(Bq, C, L)
    kt = jnp.transpose(k, (0, 2, 3, 1)).reshape(Bq, C, L)
    vt = jnp.transpose(v, (0, 2, 3, 1)).reshape(Bq, C, L)

    # mean over channels of per-channel circular correlation, via DFT:
    # corr_c = IDFT(Q_c * conj(K_c));  mean_value = IDFT(mean_c Q_c K_c*)
    Qr, Qi = _fwd_dft(qt, L)
    Kr, Ki = _fwd_dft(kt, L)
    Pr = jnp.mean(Qr * Kr + Qi * Ki, axis=1)
    Pi = jnp.mean(Qi * Kr - Qr * Ki, axis=1)
    mean_value = _inv_dft_real(Pr, Pi, L)  # [Bq, L]

    top_k = int(factor * math.log(L))
    # iterative top-k via masked max (distinct values -> identical to lax.top_k)
    iota = jnp.arange(L)[None, :]
    mm = mean_value
    weights = []
    delays = []
    for _ in range(top_k):
        wv = jnp.max(mm, axis=-1)  # [Bq]
        ai = jnp.argmax(mm, axis=-1)  # [Bq]
        weights.append(wv)
        delays.append(ai)
        mm = jnp.where(iota == ai[:, None], -jnp.inf, mm)
    weights = jnp.stack(weights, axis=-1)  # [Bq, top_k]
    delays = jnp.stack(delays, axis=-1)  # [Bq, top_k] int
    tmp_corr = jax.nn.softmax(weights, axis=-1)

    vcat = jnp.concatenate([vt, vt], axis=-1)  # [Bq, C, 2L]
    aggs = []
    for b in range(Bq):
        acc = jnp.zeros((C, L), vt.dtype)
        for i in range(top_k):
            sl = jax.lax.dynamic_slice(vcat[b], (0, delays[b, i]), (C, L))
            acc = acc + sl * tmp_corr[b, i]
        aggs.append(acc)
    agg = jnp.stack(aggs, axis=0)  # [Bq, C, L]
    agg = agg.reshape(Bq, H, E, L)
    return jnp.transpose(agg, (0, 3, 1, 2))  # [Bq, L, H, E]


def autocorr_layer(xq, xk, xv, p, factor):
    Bq, Lq, _ = xq.shape
    Lk = xk.shape[1]
    E = D_MODEL // N_HEADS
    q = (xq @ p["wq"] + p["bq"]).reshape(Bq, Lq, N_HEADS, E)
    k = (xk @ p["wk"] + p["bk"]).reshape(Bq, Lk, N_HEADS, E)
    v = (xv @ p["wv"] + p["bv"]).reshape(Bq, Lk, N_HEADS, E)
    out = autocorrelation(q, k, v, factor).reshape(Bq, Lq, D_MODEL)
    return out @ p["wo"] + p["bo"]


def encoder_layer(x, p):
    x = x + autocorr_layer(x, x, x, p["attn"], FACTOR)
    x, _ = series_decomp(x, MOVING_AVG)
    y = jax.nn.gelu(x @ p["w1"]) @ p["w2"]
    x, _ = series_decomp(x + y, MOVING_AVG)
    return x


def conv_layer(x, p):
    y = jax.nn.elu(conv1d(x, p["w"], p["b"], pad=1))
    yp = jnp.pad(y, ((0, 0), (1, 1), (0, 0)), constant_values=-jnp.inf)
    return jax.lax.reduce_window(
        yp, -jnp.inf, jax.lax.max, (1, 3, 1), (1, 2, 1), "VALID"
    )


def seasonal_norm(x, p):
    mu = x.mean(-1, keepdims=True)
    var = x.var(-1, keepdims=True)
    xh = (x - mu) / jnp.sqrt(var + 1e-5) * p["g"] + p["b"]
    return xh - xh.mean(axis=1, keepdims=True)


def decoder_layer(x, cross, p):
    x, t1 = series_decomp(x + autocorr_layer(x, x, x, p["self"], FACTOR), MOVING_AVG)
    x, t2 = series_decomp(
        x + autocorr_layer(x, cross, cross, p["cross"], FACTOR), MOVING_AVG
    )
    y = jax.nn.gelu(x @ p["w1"]) @ p["w2"]
    x, t3 = series_decomp(x + y, MOVING_AVG)
    rt = conv1d(t1 + t2 + t3, p["trend_w"], pad=1)
    return x, rt


def forward(x_enc, x_mark_enc, x_dec, x_mark_dec, params):
    mean = jnp.repeat(
        jnp.mean(x_enc, axis=1, keepdims=True), PRED_LEN - LABEL_LEN, axis=1
    )
    zeros = jnp.zeros(
        (x_dec.shape[0], PRED_LEN - LABEL_LEN, x_enc.shape[2]), x_enc.dtype
    )
    seasonal_init, trend_init = series_decomp(x_enc, MOVING_AVG)
    trend_init = jnp.concatenate([trend_init[:, :LABEL_LEN], mean], axis=1)
    seasonal_init = jnp.concatenate([seasonal_init[:, :LABEL_LEN], zeros], axis=1)
    enc = conv1d(x_enc, params["enc_emb_w"], pad=1) + x_mark_enc @ params["enc_mark_w"]
    for i in range(E_LAYERS):
        enc = encoder_layer(enc, params["enc_layers"][i])
        if i < E_LAYERS - 1:
            enc = conv_layer(enc, params["enc_conv"])
    enc = seasonal_norm(enc, params["norm"])
    dec = (
        conv1d(seasonal_init, params["dec_emb_w"], pad=1)
        + x_mark_dec @ params["dec_mark_w"]
    )
    trend = trend_init
    x = dec
    for p in params["dec_layers"]:
        x, rt = decoder_layer(x, enc, p)
        trend = trend + rt
    x = seasonal_norm(x, params["norm"])
    seasonal = x @ params["proj_w"] + params["proj_b"]
    out = (trend + seasonal) @ params["comp_w"] + params["comp_b"]
    return out[:, -PRED_LEN:]


_jit_forward = jax.jit(forward)
_pmap_forward = None


def _kernel_pmap(x_enc, x_mark_enc, x_dec, x_mark_dec, params, devs):
    """One compiled module for all 8 replicas, single dispatch (preferred)."""
    global _pmap_forward
    if _pmap_forward is None:
        _pmap_forward = jax.pmap(forward, in_axes=(0, 0, 0, 0, None), devices=devs)
    n = len(devs)
    rs = lambda a: a.reshape((n, a.shape[0] // n) + a.shape[1:])
    out = np.asarray(
        _pmap_forward(rs(x_enc), rs(x_mark_enc), rs(x_dec), rs(x_mark_dec), params)
    )
    return out.reshape((out.shape[0] * out.shape[1],) + out.shape[2:])


def _kernel_jit_loop(x_enc, x_mark_enc, x_dec, x_mark_dec, params, devs):
    """Fallback: per-device jit executables, async dispatch overlaps cores."""
    shard = x_enc.shape[0] // len(devs)
    futs = []
    for i, d in enumerate(devs):
        sl = slice(i * shard, (i + 1) * shard)
        args = (
            jax.device_put(x_enc[sl], d),
            jax.device_put(x_mark_enc[sl], d),
            jax.device_put(x_dec[sl], d),
            jax.device_put(x_mark_dec[sl], d),
            jax.device_put(params, d),
        )
        futs.append(_jit_forward(*args))
    return np.concatenate([np.asarray(f) for f in futs], axis=0)


def kernel(x_enc, x_mark_enc, x_dec, x_mark_dec, params):
    devs = [d for d in jax.devices() if d.platform != "cpu"][:N_CORES]
    x_enc = np.asarray(x_enc)
    x_mark_enc = np.asarray(x_mark_enc)
    x_dec = np.asarray(x_dec)
    x_mark_dec = np.asarray(x_mark_dec)
    params = jax.tree.map(np.asarray, params)
    try:
        out = _kernel_pmap(x_enc, x_mark_enc, x_dec, x_mark_dec, params, devs)
    except Exception:
        out = _kernel_jit_loop(x_enc, x_mark_enc, x_dec, x_mark_dec, params, devs)
    return out.astype(np.float32)
